# revision 4
# baseline (speedup 1.0000x reference)
"""Multi-head causal self-attention (B=2, S=2048, D=2048, H=16) on 8 TRN2 cores.

Sharding: data parallel on batch (2) x tensor parallel on head groups (4 heads
per core). Each core computes QKV projections for its 512 q/k/v channels, the
causal attention for its 4 heads, and a partial output projection against its
512 columns of Wo. The host sums the 4 partials per batch and adds bo.

All matmul operands are fp16 (full PE rate) with fp32 PSUM accumulation;
softmax statistics stay fp32. Layout/scheduling:
- x^T is DMA'd once into SBUF as 64 [128, 512] column-chunks, sg-major so
  arrival order matches the qk pass's consumption order; the v pass reuses
  them as stationary slices (no second HBM stream). The first chunk and the
  first wqk tile are split into smaller DMAs so the first matmul's
  dependencies land early.
- One PSUM pool with manual bank tags spans all phases (no pool-transition
  barrier); q/k use banks 0-3/4-7, v alternates by sg parity, attention
  scores rotate banks 0-3 and po/sm/po3 share banks 4-7.
- Softmax row sums come from a per-unit fp16 accumulation of the exp'd
  tiles on DVE (serial in-place adds) followed by ONE ones-matmul on the
  accumulator -- removing the per-tile ones-matmuls from the PE stream.
- The v bias folds in after normalization (attn = po/sm + bv per head), so
  the v pass has no bias matmuls.
- Scores are computed in [k, q] orientation so exp'd tiles feed the PV matmul
  as the moving operand; the causal mask for diagonal tiles is a matmul
  accumulation (maskT.T @ I), keeping the score->exp->PV chain off DVE.
- The attention inner loop is software-pipelined (PV lags scores by 3 tiles);
  each unit's softmax finalize (ones-matmul + reciprocal + normalize + bias)
  is deferred into the next unit's stream so the PE never waits on the DVE
  accumulation chain.
- Output-projection (phase 3) work items are interleaved into the attention
  units' slots at a paced rate: they backfill the PE while ACT works through
  the exps (scores+PV alone are shorter per tile than the exp), and levels
  become eligible a few slots after their last head finalizes. Partials are
  staged fp16 and written [128, 2048] per row; the host accumulates in fp32.
- wo is loaded into SBUF space aliased over freed x-chunk buffers, which
  defers its HBM fetch past the projection phase's bandwidth-critical window.
"""

import math
from contextlib import ExitStack

import numpy as np

import concourse.bass as bass
import concourse.tile as tile
from concourse import bacc, mybir
from concourse.bass_utils import run_bass_kernel_spmd

B, S, D, H, HD = 2, 2048, 2048, 16, 128
N_CORES = 8
HPC = 4          # heads per core
HJ = HPC * HD    # 512 projection channels per core
SG = 512         # column-group width for matmuls
ND = D // 128    # 16 contraction tiles over model dim
NS = S // 128    # 16 tiles over sequence
NG = S // SG     # 4 column groups over sequence

F32 = mybir.dt.float32
F16 = mybir.dt.float16
MUL = mybir.AluOpType.mult
ADD = mybir.AluOpType.add
EXP = mybir.ActivationFunctionType.Exp

last_exec_time_ns = None


def _build():
    nc = bacc.Bacc("TRN2", target_bir_lowering=False, debug=False)

    xt = nc.dram_tensor("xt", [D, S], F16, kind="ExternalInput").ap()
    wqk = nc.dram_tensor("wqk", [D, 2 * HJ], F16, kind="ExternalInput").ap()
    wv = nc.dram_tensor("wv", [D, HJ], F16, kind="ExternalInput").ap()
    wo = nc.dram_tensor("wo", [HJ, D], F16, kind="ExternalInput").ap()
    bq = nc.dram_tensor("bq", [HJ, 1], F32, kind="ExternalInput").ap()
    bk = nc.dram_tensor("bk", [HJ, 1], F32, kind="ExternalInput").ap()
    bv = nc.dram_tensor("bv", [HJ, 1], F32, kind="ExternalInput").ap()
    maskt = nc.dram_tensor("maskt", [128, 128], F16, kind="ExternalInput").ap()
    ident = nc.dram_tensor("ident", [128, 128], F16, kind="ExternalInput").ap()
    out = nc.dram_tensor("out", [S, D], F16, kind="ExternalOutput").ap()

    with tile.TileContext(nc) as tc, ExitStack() as es:
        cpool = es.enter_context(tc.tile_pool(name="const", bufs=1))
        rpool = es.enter_context(tc.tile_pool(name="res", bufs=1))
        # small attention-phase pools allocated before the big phase-1 pools
        # so their SBUF never overlaps x chunks still being read by the v
        # pass when the first exp fires
        etpool = es.enter_context(tc.tile_pool(name="et", bufs=6))
        accpool = es.enter_context(tc.tile_pool(name="acc", bufs=2))
        rrpool = es.enter_context(tc.tile_pool(name="rr", bufs=2))
        # single PSUM pool for the whole kernel: manual bank tags bk0..bk7
        ps = es.enter_context(tc.tile_pool(name="ps", bufs=1, space="PSUM"))

        # constants on the gpsimd queue: tiny and off the x/w stream path
        onesm_sb = cpool.tile([128, 128], F16, name="onesm_sb", tag="onesm")
        nc.gpsimd.memset(onesm_sb[:], 1.0)
        bq_sb = []
        bk_sb = []
        bv_sb = []
        for i in range(HPC):
            t = cpool.tile([128, 1], F32, name=f"bq{i}", tag=f"bq{i}")
            nc.gpsimd.dma_start(t[:], bq[i * 128:(i + 1) * 128, :])
            bq_sb.append(t)
            t = cpool.tile([128, 1], F32, name=f"bk{i}", tag=f"bk{i}")
            nc.gpsimd.dma_start(t[:], bk[i * 128:(i + 1) * 128, :])
            bk_sb.append(t)
            t = cpool.tile([128, 1], F32, name=f"bv{i}", tag=f"bv{i}")
            nc.gpsimd.dma_start(t[:], bv[i * 128:(i + 1) * 128, :])
            bv_sb.append(t)
        maskt_sb = cpool.tile([128, 128], F16, name="maskt", tag="maskt")
        nc.gpsimd.dma_start(maskt_sb[:], maskt[:])
        ident_sb = cpool.tile([128, 128], F16, name="ident", tag="ident")
        nc.gpsimd.dma_start(ident_sb[:], ident[:])

        qT = [rpool.tile([128, S], F16, name=f"qT{i}", tag=f"qT{i}")
              for i in range(HPC)]
        kT = [rpool.tile([128, S], F16, name=f"kT{i}", tag=f"kT{i}")
              for i in range(HPC)]
        vsb = [rpool.tile([128, HJ], F16, name=f"v{j}", tag=f"v{j}")
               for j in range(NS)]

        # ---------------- phase 1: q/k/v projections ----------------------
        # x^T d-tiles live in SBUF for the whole phase: moving operand for
        # the qk pass, stationary slices for the v pass.
        with tc.tile_pool(name="xf", bufs=1) as xfpool, \
             tc.tile_pool(name="wqk", bufs=1) as wpool, \
             tc.tile_pool(name="wvp", bufs=1) as wvpool:
            xcol = [[None] * NG for _ in range(ND)]
            wtile = {}
            for sg in range(NG):
                for d in range(ND):
                    txc = xfpool.tile([128, SG], F16, name=f"xc{d}_{sg}",
                                      tag=f"xc{d}_{sg}")
                    if sg == 0 and d == 0:
                        # split across both queues so the first matmul's
                        # moving operand lands ~2x sooner
                        nc.sync.dma_start(txc[:, :256], xt[0:128, 0:256])
                        nc.scalar.dma_start(txc[:, 256:512], xt[0:128, 256:512])
                    else:
                        q_ = nc.sync if d % 2 == 0 else nc.scalar
                        q_.dma_start(txc[:], xt[d * 128:(d + 1) * 128,
                                                sg * SG:(sg + 1) * SG])
                    xcol[d][sg] = txc
                    if sg == 0:
                        tw = wpool.tile([128, 2 * HJ], F16, name=f"wqk{d}",
                                        tag=f"wqk{d}")
                        if d == 0:
                            # head slice first: the very first matmul only
                            # needs wqk[0][:, :128]
                            nc.scalar.dma_start(tw[:, :128], wqk[0:128, :128])
                            nc.scalar.dma_start(tw[:, 128:], wqk[0:128, 128:])
                        else:
                            qw = nc.scalar if d % 2 == 0 else nc.sync
                            qw.dma_start(tw[:], wqk[d * 128:(d + 1) * 128, :])
                        wtile[("q", d)] = tw[:, :HJ]
                        wtile[("k", d)] = tw[:, HJ:]
            wvt = {}
            for d in range(ND):
                # gpsimd queue: keeps the HWDGE queues clear for the x/wq/wk
                # stream that feeds the first sg group; wv isn't needed until
                # the v pass
                t = wvpool.tile([128, HJ], F16, name=f"wv{d}", tag=f"wv{d}")
                nc.gpsimd.dma_start(t[:], wv[d * 128:(d + 1) * 128, :])
                wvt[d] = t

            for sg in range(NG):
                psg = {}
                for i in range(HPC):
                    psg[("q", i)] = ps.tile([128, SG], F32, name=f"psa{i}",
                                            tag=f"bk{i}")
                for i in range(HPC):
                    psg[("k", i)] = ps.tile([128, SG], F32, name=f"psb{i}",
                                            tag=f"bk{4 + i}")
                for d in range(ND):
                    xs = xcol[d][sg][:]
                    for which in ("q", "k"):
                        for i in range(HPC):
                            nc.tensor.matmul(
                                psg[(which, i)][:],
                                lhsT=wtile[(which, d)][:, i * 128:(i + 1) * 128],
                                rhs=xs,
                                start=(d == 0), stop=(d == ND - 1))
                for i in range(HPC):
                    # drains on DVE (ACT's activation-table copies are slow)
                    nc.vector.tensor_scalar_add(
                        qT[i][:, sg * SG:(sg + 1) * SG], psg[("q", i)][:],
                        bq_sb[i][:])
                    nc.vector.tensor_scalar_add(
                        kT[i][:, sg * SG:(sg + 1) * SG], psg[("k", i)][:],
                        bk_sb[i][:])

            # v pass: stationary x^T slices, moving Wv^T; psum banks alternate
            # by sg parity for cross-sg overlap. No bias matmul: bv is folded
            # in after softmax normalization (exact: sum of weights is 1).
            for sg in range(NG):
                base = 0 if sg % 2 == 0 else 4
                psv = [ps.tile([128, HJ], F32, name=f"psv{i}",
                               tag=f"bk{base + i}")
                       for i in range(4)]
                for d in range(ND):
                    for ss in range(4):
                        nc.tensor.matmul(
                            psv[ss][:],
                            lhsT=xcol[d][sg][:, ss * 128:(ss + 1) * 128],
                            rhs=wvt[d][:],
                            start=(d == 0), stop=(d == ND - 1))
                for ss in range(4):
                    nc.vector.tensor_copy(vsb[sg * 4 + ss][:], psv[ss][:])

        # ---------------- phases 2+3: attention + output projection -------
        # Scores in [k, q] orientation; exp'd tiles feed PV as the moving
        # operand. Row sums: DVE accumulates the exp'd tiles per unit in
        # fp16, then a single ones-matmul per unit reduces the accumulator
        # across partitions. Phase 3 items interleave into the unit slots.
        with tc.tile_pool(name="wo2", bufs=1) as wopool, \
             tc.tile_pool(name="attnp", bufs=1) as apool, \
             tc.tile_pool(name="stg", bufs=3) as stpool:
            # wo aliases freed x-chunk SBUF: its DMA naturally defers until
            # the v pass has consumed those chunks, off the critical window
            wot = []
            for t_ in range(HPC):
                wt = wopool.tile([128, D], F16, name=f"wo{t_}", tag=f"wo{t_}")
                nc.gpsimd.dma_start(wt[:], wo[t_ * 128:(t_ + 1) * 128, :])
                wot.append(wt)
            attn = [apool.tile([128, S], F16, name=f"at{h}", tag=f"at{h}")
                    for h in range(HPC)]

            LAG = 3
            state = {"psc": 0, "po3": 0, "credit": 0.0}
            pending = []     # phase-3 items (lvl, st, dg) ready to emit
            stage_map = {}   # st -> [stage tile, drained-count]

            def emit_finalize(fin):
                g, h, po, acc = fin
                smp = ps.tile([128, SG], F32, name="smp", tag="bk6")
                nc.tensor.matmul(smp[:], lhsT=onesm_sb[:], rhs=acc[:],
                                 start=True, stop=True)
                rr = rrpool.tile([128, SG], F32, name="rr", tag="rr")
                # ~18 correct bits, 5x faster than reciprocal(); sums of exp
                # are in [1, 4e3] so no edge cases
                nc.vector.reciprocal_approx_fast(rr[:], smp[:])
                sl = attn[h][:, g * SG:(g + 1) * SG]
                nc.vector.tensor_tensor(sl, po[:], rr[:], op=MUL)
                nc.vector.tensor_scalar_add(sl, sl, bv_sb[h][:])

            def emit_item(lvl, st, dg):
                ent = stage_map.get(st)
                if ent is None:
                    stage = stpool.tile([128, D], F16, name="stg", tag="stg")
                    ent = stage_map[st] = [stage, 0]
                stage = ent[0]
                po3 = ps.tile([128, SG], F32, name="po3",
                              tag=f"bk{6 + state['po3'] % 2}")
                state["po3"] += 1
                for hh in range(HPC):
                    nc.tensor.matmul(
                        po3[:],
                        lhsT=attn[hh][:, st * 128:(st + 1) * 128],
                        rhs=wot[hh][:, dg * SG:(dg + 1) * SG],
                        start=(hh == 0), stop=(hh == HPC - 1))
                nc.vector.tensor_copy(stage[:, dg * SG:(dg + 1) * SG], po3[:])
                ent[1] += 1
                if ent[1] == 4:
                    if lvl == 0 and st == 0:
                        nc.sync.dma_start(out[st * 128:(st + 1) * 128, :D // 2],
                                          stage[:, :D // 2])
                        nc.scalar.dma_start(out[st * 128:(st + 1) * 128, D // 2:],
                                            stage[:, D // 2:])
                    else:
                        nc.sync.dma_start(out[st * 128:(st + 1) * 128, :],
                                          stage[:])
                    del stage_map[st]

            def level_items(lvl):
                return [(lvl, st, dg)
                        for st in range(4 * lvl + 3, 4 * lvl - 1, -1)
                        for dg in range(NG)]

            units = [(g, h) for g in range(NG - 1, -1, -1)
                     for h in range(HPC)]
            prev_fin = None
            ready_level = None
            for u, (g, h) in enumerate(units):
                nkt = 4 * g + 4
                po = ps.tile([128, SG], F32, name="po", tag=f"bk{4 + u % 2}")
                acc = accpool.tile([128, SG], F16, name="acc", tag="acc")
                pend = {}
                for i in range(nkt + LAG):
                    if i == 2 and prev_fin is not None:
                        fin = prev_fin
                        prev_fin = None
                        emit_finalize(fin)
                        if fin[1] == HPC - 1:
                            ready_level = fin[0]
                    if i == 6 and ready_level is not None:
                        # a few slots after the level's last finalize so its
                        # DVE normalize has landed before phase-3 reads it
                        pending.extend(level_items(ready_level))
                        ready_level = None
                    if i < nkt:
                        kt = i
                        qoff = max(0, kt - 4 * g) * 128
                        w = SG - qoff
                        diag = kt >= 4 * g
                        psc = ps.tile([128, SG], F32, name="psc",
                                      tag=f"bk{state['psc'] % 4}")
                        state["psc"] += 1
                        nc.tensor.matmul(
                            psc[:, :w],
                            lhsT=kT[h][:, kt * 128:(kt + 1) * 128],
                            rhs=qT[h][:, g * SG + qoff:(g + 1) * SG],
                            start=True, stop=not diag)
                        if diag:
                            # diagonal block = this tile's first 128 cols
                            nc.tensor.matmul(
                                psc[:, 0:128], lhsT=maskt_sb[:],
                                rhs=ident_sb[:], start=False, stop=True)
                        et = etpool.tile([128, SG], F16, name="et", tag="et")
                        nc.scalar.activation(et[:, :w], psc[:, :w], EXP)
                        # acc chain on gpsimd (SBUF-only, so legal there):
                        # keeps DVE free for psum drains + finalize
                        if i == 0:
                            # first tile is always full width
                            nc.gpsimd.tensor_copy(acc[:], et[:])
                        else:
                            nc.gpsimd.tensor_tensor(
                                acc[:, qoff:], acc[:, qoff:], et[:, :w],
                                op=ADD)
                        pend[i] = (et, w, qoff, kt)
                    if i >= LAG:
                        et, w, qoff, kt = pend.pop(i - LAG)
                        nc.tensor.matmul(
                            po[:, qoff:],
                            lhsT=vsb[kt][:, h * 128:(h + 1) * 128],
                            rhs=et[:, :w],
                            start=(i - LAG == 0), stop=(i - LAG == nkt - 1))
                    state["credit"] += 0.45
                    if state["credit"] >= 1.0 and pending:
                        state["credit"] -= 1.0
                        emit_item(*pending.pop(0))
                prev_fin = (g, h, po, acc)

            emit_finalize(prev_fin)
            pending.extend(level_items(0))
            for it in pending:
                emit_item(*it)

    nc.finalize()
    return nc


_NC_CACHE = []


def kernel(hidden_states, Wq, bq, Wk, bk, Wv, bv, Wo, bo, **_unused):
    global last_exec_time_ns

    hidden_states = np.asarray(hidden_states, dtype=np.float32)
    Wq = np.asarray(Wq, dtype=np.float32)
    Wk = np.asarray(Wk, dtype=np.float32)
    Wv = np.asarray(Wv, dtype=np.float32)
    Wo = np.asarray(Wo, dtype=np.float32)
    bq = np.asarray(bq, dtype=np.float32)
    bk = np.asarray(bk, dtype=np.float32)
    bv = np.asarray(bv, dtype=np.float32)
    bo = np.asarray(bo, dtype=np.float32)

    if not _NC_CACHE:
        _NC_CACHE.append(_build())
    nc = _NC_CACHE[0]

    scale = 1.0 / math.sqrt(HD)
    q_idx = np.arange(128)[:, None]   # [q, 1]
    k_idx = np.arange(128)[None, :]   # [1, k]
    # maskT in [q, k] orientation: -50 where k > q (strict upper triangle)
    maskt = np.where(k_idx > q_idx, -50.0, 0.0).astype(np.float16)
    ident = np.eye(128, dtype=np.float16)

    xts = [np.ascontiguousarray(hidden_states[b].T).astype(np.float16)
           for b in range(B)]
    in_maps = []
    for c in range(N_CORES):
        b, hg = divmod(c, HPC)
        sl = slice(hg * HJ, (hg + 1) * HJ)
        in_maps.append({
            "xt": xts[b],
            "wqk": np.ascontiguousarray(
                np.concatenate([(Wq[sl] * scale).T, Wk[sl].T],
                               axis=1)).astype(np.float16),
            "wv": np.ascontiguousarray(Wv[sl].T).astype(np.float16),
            "wo": np.ascontiguousarray(Wo[:, sl].T).astype(np.float16),
            "bq": (bq[sl] * scale).reshape(HJ, 1).copy(),
            "bk": bk[sl].reshape(HJ, 1).copy(),
            "bv": bv[sl].reshape(HJ, 1).copy(),
            "maskt": maskt,
            "ident": ident,
        })

    res = run_bass_kernel_spmd(nc, in_maps, core_ids=list(range(N_CORES)))
    last_exec_time_ns = res.exec_time_ns

    outp = np.empty((B, S, D), np.float32)
    for b in range(B):
        acc = res.results[b * HPC]["out"].astype(np.float32)
        for c in range(b * HPC + 1, (b + 1) * HPC):
            acc = acc + res.results[c]["out"].astype(np.float32)
        outp[b] = acc + bo[None, :]
    return outp
